# revision 34
# baseline (speedup 1.0000x reference)
"""KAN expert kernel for Trainium2 (8 NeuronCores, data-parallel over batch).

Math: out[b,j] = sum_{i,g} basis_g(x[b,i]) * coeff[i,j,g] * scaling[i,j]
with cubic B-spline basis on the uniform extended grid g_m = -1 + 0.4*m.

Truncated-power identity: basis_g(x) = (1/(6h^3)) * sum_r w_r relu(x-g_{g+r})^3
with w = [1,-4,6,-4,1]; for x in [-1,1) only m=0..4 survive, and the linear
combine folds into host-side weights C'[m,i,j].  Each core computes
Q_m = relu(x - g_m)^3 and one [512b x 2560k]@[2560k x 512j] matmul (float32r,
full PE stream rate at >=256 moving rows).

Schedule (per core):
  - host pre-swizzles x and C' into partition-major [128, cols] slabs so
    every DMA is a plain 2D row-contiguous transfer.
  - all input DMAs issue from the otherwise-idle SP engine (HWDGE; one
    completion semaphore per dma_start): X first (3 pieces, smallest
    first), then W in 10 half-tiles in consumption order.
  - features: ACT computes s_m = (x-g_m)^2 and the hi-half relu straight
    from X (bias trick), DVE does the lo-half relu + q-mults, GpSimd only
    the hi-half q-mults (its dual-op tensor_scalar is ~20x slow).
  - a scratch-psum warmup keeps the PE p-state hot while features are
    computed, so the real matmuls stream at full rate from the start.
  - matmul groups run lo-leading-hi by one channel; psum tiles are evicted
    (DVE copy) and DMA'd out per-bc as each finishes accumulating.
  - single-sync-wait discipline (Walrus limit): 1-elem W probes ordered
    before each q-mult make the producer tick imply weight arrival; a
    post-schedule pass prunes the then-redundant waits.
"""

import numpy as np

BATCH = 4096
IN_DIM = 512
OUT_DIM = 512
GRID_SIZE = 5
K = 3
N_CORES = 8
P = 128
NM = 5                      # relu^3 feature channels
BC = BATCH // N_CORES       # 512 batch rows per core
NIC = IN_DIM // P           # 4 input-dim chunks
NCOL = NIC * BC             # 2048 feature columns per partition row
SPLIT = 1024                # DVE owns cols [0:SPLIT), GpSimd [SPLIT:NCOL)

_W_BINOM = np.array([1.0, -4.0, 6.0, -4.0, 1.0])

_cached = {}


def _grid_f32():
    h = 2.0 / GRID_SIZE
    return np.float32(-1.0 + h * np.arange(GRID_SIZE + 2 * K + 1))


def _build_nc():
    import concourse.bass as bass
    import concourse.mybir as mybir
    from concourse.tile import TileContext

    class LeanTailTileContext(TileContext):
        """Drop the second exit all-engine barrier: the semaphore clears run
        on GpSimd's stream after the first barrier, and nothing executes
        after them until the next NEFF launch (which starts only once every
        engine stream has ended), so the closing barrier only adds ~4us."""

        def _drain_and_barrier(self, tick_clock, wait_clock):
            from concourse.tile import ScopedClock
            drain_inst = self.nc.sync.drain()
            wait_clock.add_sem_waits(
                drain_inst.ins, ScopedClock({None: tick_clock.global_clock}))
            self.nc.all_engine_barrier()
            assert self.sems is not None
            popped = self.nc._tile_sem_poison_stack.pop()
            assert popped is self._sem_poison
            self.nc.clear_and_free_semaphores(
                list(self.sems.allocated().values()))

    dt = mybir.dt
    grid = _grid_f32()

    nc = bass.Bass()
    xt = nc.dram_tensor("xt", [P, NCOL], dt.float32, kind="ExternalInput")
    cw = nc.dram_tensor("cw", [P, NM * NCOL], dt.float32r,
                        kind="ExternalInput")
    # bf16 output: halves the out-DMA bytes and uses DVE's 4x bf16 copy
    # path for psum eviction; the host upcasts after gather.  Quantization
    # adds ~2^-9 relative, well inside the error budget.
    out = nc.dram_tensor("out", [BC, OUT_DIM], dt.bfloat16,
                         kind="ExternalOutput")

    # Register the ACT biases (-g_m) as preamble const APs (memset +
    # all-engine barrier, exactly like Bass's built-in consts) so the
    # squares carry no bias-related sync waits.
    for m in range(NM):
        v = float(np.float32(-grid[m]))
        if (dt.float32, v) not in nc.const_aps.aps:
            t = nc.alloc_sbuf_tensor(f"const-bias-{m}", [P, 1], dt.float32)
            nc.gpsimd.memset(t.ap(), v)
            nc.const_aps.aps[(dt.float32, v)] = t.ap()
    nc.all_engine_barrier()

    with LeanTailTileContext(nc) as tc:
        with tc.tile_pool(name="main", bufs=1) as pool, \
             tc.tile_pool(name="psum", bufs=1, space="PSUM") as psum_pool:
            X = pool.tile([P, NCOL], dt.float32, tag="X")
            # weight tiles split per engine half: Wl[m] holds ic 0..1,
            # Wh[m] holds ic 2..3 (1024 columns each)
            Wl = [pool.tile([P, SPLIT], dt.float32r, tag=f"Wl{m}",
                            name=f"Wl{m}") for m in range(NM)]
            Wh = [pool.tile([P, NCOL - SPLIT], dt.float32r, tag=f"Wh{m}",
                            name=f"Wh{m}") for m in range(NM)]
            # All input DMAs share SP's HWDGE queue (inbound bandwidth is a
            # shared resource — splitting across queues only reorders it), in
            # just-in-time consumption order: first X quarter, then the first
            # lo-weight group (unblocks the first matmuls), the rest of X,
            # then alternating weight groups.
            QTR = SPLIT // 2

            def dma_wl(m):
                nc.sync.dma_start(out=Wl[m][:],
                                  in_=cw[:, m * NCOL:m * NCOL + SPLIT])

            def dma_wh(m):
                nc.sync.dma_start(out=Wh[m][:],
                                  in_=cw[:, m * NCOL + SPLIT:(m + 1) * NCOL])

            # X fully first (features and warmups never starve), then the
            # weight half-tiles in exact matmul consumption order
            nc.sync.dma_start(out=X[:, 0:QTR], in_=xt[:, 0:QTR])
            nc.sync.dma_start(out=X[:, QTR:SPLIT], in_=xt[:, QTR:SPLIT])
            nc.sync.dma_start(out=X[:, SPLIT:NCOL], in_=xt[:, SPLIT:NCOL])
            for m in range(NM):
                dma_wl(m)
                dma_wh(m)

            s_lo = [pool.tile([P, SPLIT], dt.float32, tag=f"sl{m}", name=f"sl{m}")
                    for m in range(NM)]
            s_hi = [pool.tile([P, NCOL - SPLIT], dt.float32, tag=f"sh{m}", name=f"sh{m}")
                    for m in range(NM)]
            q_lo = [pool.tile([P, SPLIT], dt.float32r, tag=f"ql{m}", name=f"ql{m}")
                    for m in range(NM)]
            q_hi = [pool.tile([P, NCOL - SPLIT], dt.float32r, tag=f"qh{m}", name=f"qh{m}")
                    for m in range(NM)]
            scr_d = pool.tile([1, 1], dt.float32r, tag="scrd")
            scr_g = pool.tile([1, 1], dt.float32r, tag="scrg")
            # GpSimd (idle this early) fills the f32r warmup operands for
            # the PE p-state warmup from the first X piece, so the warmup
            # matmuls pace themselves behind the X DMA and a single Pool
            # tick covers both operands.
            wu_lhs = pool.tile([P, 1], dt.float32r, tag="wulhs", name="wulhs")
            wu_rhs = pool.tile([P, OUT_DIM], dt.float32r, tag="wurhs",
                               name="wurhs")
            nc.gpsimd.tensor_copy(wu_lhs[:], X[:, 0:1])
            wuc = nc.gpsimd.tensor_copy(wu_rhs[:], X[:, 0:OUT_DIM])

            from concourse.bass import _add_dep_helper
            Sq = mybir.ActivationFunctionType.Square
            Re = mybir.ActivationFunctionType.Relu

            def bias_of(m):
                return float(np.float32(-grid[m]))

            # ACT owns both squares and the hi-half relu so each q-mult has
            # a single (ACT) cross-engine wait.  GpSimd's dual-op
            # tensor_scalar is pathologically slow, so it only ever runs
            # tensor_tensor muls.  The lo-half squares run one channel ahead
            # of the hi-half ops, mirroring the lo-leads-hi matmul order.
            # Channel 0's lo half is split into two column pieces matching
            # the X DMA pieces so the first matmuls fire as soon as the
            # first X quarter lands.
            def lo_pieces(m):
                return [(0, QTR), (QTR, SPLIT)] if m == 0 else [(0, SPLIT)]

            for (a, b) in lo_pieces(0):
                nc.scalar.activation(s_lo[0][:, a:b], X[:, a:b], Sq,
                                     bias=bias_of(0))
            nc.scalar.activation(s_lo[1][:], X[:, 0:SPLIT], Sq,
                                 bias=bias_of(1))
            for m in range(NM):
                bv = bias_of(m)
                nc.scalar.activation(s_hi[m][:], X[:, SPLIT:NCOL], Sq,
                                     bias=bv)
                nc.scalar.activation(q_hi[m][:], X[:, SPLIT:NCOL], Re,
                                     bias=bv)
                if m + 2 < NM:
                    nc.scalar.activation(s_lo[m + 2][:], X[:, 0:SPLIT], Sq,
                                         bias=bias_of(m + 2))

            # DVE: relu + q-mult for the lo half, channel by channel.
            # 1-elem probes of each m-group's weights precede the q-mult in
            # each producer's program order (enforced with explicit nosync
            # dep edges so the scheduler cannot hoist the mult), so a matmul
            # waiting on the producer's tick transitively knows W_m has
            # landed (keeps every matmul at a single sync wait).
            for m in range(NM):
                gm = float(grid[m])
                pr_d = None
                for (a, b) in lo_pieces(m):
                    nc.vector.tensor_scalar(
                        q_lo[m][:, a:b], X[:, a:b], gm, 0.0,
                        mybir.AluOpType.subtract, mybir.AluOpType.max)
                    if pr_d is None:
                        # probe sits after the relu so the relu overlaps the
                        # weight DMA instead of stalling behind the probe
                        pr_d = nc.vector.tensor_copy(scr_d[0:1, 0:1],
                                                     Wl[m][0:1, 0:1])
                    ml = nc.vector.tensor_mul(
                        q_lo[m][:, a:b], q_lo[m][:, a:b], s_lo[m][:, a:b])
                    _add_dep_helper(ml.ins, pr_d.ins, sync=False,
                                    reason="Wl probe before lo q-mult")

            # GpSimd: hi-half q-mults (inputs both come from ACT)
            for m in range(NM):
                pr_g = nc.gpsimd.tensor_copy(scr_g[0:1, 0:1], Wh[m][0:1, 0:1])
                if m == 0:
                    # keep the warmup operand copies ahead of the first
                    # (weight-DMA-gated) probe in Pool program order
                    _add_dep_helper(pr_g.ins, wuc.ins, sync=False,
                                    reason="warmup copies before first probe")
                mh = nc.gpsimd.tensor_mul(q_hi[m][:], q_hi[m][:], s_hi[m][:])
                _add_dep_helper(mh.ins, pr_g.ins, sync=False,
                                reason="Wh probe before hi q-mult")

            psums = [psum_pool.tile([P, OUT_DIM], dt.float32, tag=f"ps{b}",
                                    name=f"ps{b}")
                     for b in range(BC // P)]
            O = pool.tile([P, (BC // P) * OUT_DIM], dt.bfloat16, tag="O")
            out_dmas = []
            n_k = NM * NIC
            f32r = dt.float32r
            # PE p-state warmup: the Tensor engine only reaches full clock
            # after ~3us of continuous work, and a stalled PE falls back to
            # mid-state (~2.8x slower streaming).  Run a few throwaway
            # matmuls into a scratch psum bank while the features are being
            # computed; each reads a freshly-squared ACT output so they pace
            # themselves behind the X DMA instead of burning out early.
            wu_ps = psum_pool.tile([1, OUT_DIM], dt.float32, tag="wups",
                                   name="wups")
            for k in range(8):
                nc.tensor.matmul(wu_ps[:], wu_lhs[:], wu_rhs[:],
                                 start=True, stop=True)

            cnt = [0] * (BC // P)

            def emit_mm(m, ic, bc):
                cnt[bc] += 1
                c0 = ic * BC + bc * P
                if c0 < SPLIT:
                    lhsT = q_lo[m][:, c0:c0 + P]
                    rhs = Wl[m][:, ic * BC:(ic + 1) * BC]
                else:
                    lhsT = q_hi[m][:, c0 - SPLIT:c0 - SPLIT + P]
                    rhs = Wh[m][:, (ic - 2) * BC:(ic - 1) * BC]
                nc.tensor.matmul(psums[bc][:], lhsT, rhs,
                                 start=(cnt[bc] == 1), stop=(cnt[bc] == n_k))

            # m1-lo runs between m0-lo and m0-hi: the hi-half features pass
            # through three engines (ACT square+relu, then GpSimd mult), so
            # m0-hi lands ~2us after m0-lo and would stall a strict m-order.
            order = [(0, 'lo'), (1, 'lo'), (0, 'hi'), (1, 'hi'),
                     (2, 'lo'), (2, 'hi'), (3, 'lo'), (3, 'hi'),
                     (4, 'lo'), (4, 'hi')]
            for (m, half) in order[:-1]:
                ics = (0, 1) if half == 'lo' else (2, 3)
                for ic in ics:
                    for bc in range(BC // P):
                        emit_mm(m, ic, bc)
            # final group runs bc-major so each psum finishes (and is
            # evicted + written out) while the others still accumulate
            for bc in range(BC // P):
                for ic in (2, 3):
                    emit_mm(NM - 1, ic, bc)
                nc.vector.tensor_copy(
                    O[:, bc * OUT_DIM:(bc + 1) * OUT_DIM], psums[bc][:])
                # ACT's HWDGE queue is separate from SP's input queue, so
                # the output never waits behind the last weight transfers
                od = nc.scalar.dma_start(
                    out=out[bc * P:(bc + 1) * P, :],
                    in_=O[:, bc * OUT_DIM:(bc + 1) * OUT_DIM])
                out_dmas.append(od)

            # Walrus caps sync waits per instruction; absorb all but the last
            # out-DMA completion into standalone SP nops so the final SP
            # drain needs only one wait.
            from concourse.bass import _add_dep_helper
            for od in out_dmas[:-1]:
                nop = nc.sync.nop(nofuse=True)
                _add_dep_helper(nop.ins, od.ins, sync=True,
                                reason="absorb out-dma wait before drain")

    # Walrus rejects >1 sync wait per compute instruction on this toolchain.
    # Provably redundant waits:
    #  - same-engine waits (every engine is an in-order FIFO),
    #  - matmul DMA waits (weight arrival is guaranteed through the
    #    probe -> q-mult chain the matmul already waits on).
    eng2sem = {"EngineType.DVE": ("DVE_",),
               "EngineType.Activation": ("Activation_",),
               "EngineType.PE": ("PE_",),
               "EngineType.Pool": ("Pool_",),
               "EngineType.SP": ("SP_",)}
    out_sems = set()
    for od in out_dmas:
        for u in (od.ins.sync_info.on_update or []):
            nm_ = getattr(u, "ant_name", "") or ""
            if nm_:
                out_sems.add(nm_)
    bad = []
    for blk in nc.m.functions[0].blocks:
        for inst in blk.instructions:
            si = inst.sync_info
            if si is None or not si.on_wait:
                continue
            prefs = eng2sem.get(str(inst.engine))
            keep = [w for w in si.on_wait
                    if prefs is None
                    or not (w.ant_name or "").startswith(prefs)]
            tname = type(inst).__name__
            if tname == "InstMatmult":
                eng = [w for w in keep
                       if (w.ant_name or "").startswith(("DVE_", "Pool_"))]
                if eng:
                    keep = eng
            if tname == "InstDMACopy":
                # out-DMAs: the DVE evict tick transitively dominates every
                # input-DMA completion (evict <- matmuls <- q/probes <- X,W),
                # so HWDGE ring-slot reuse waits are redundant.
                eng = [w for w in keep
                       if (w.ant_name or "").startswith("DVE_")]
                if eng:
                    keep = eng
            if tname == "InstDrain" and len(keep) > 8:
                sel = [w for w in keep if (w.ant_name or "") in out_sems]
                if sel:
                    keep = sel
            if len(keep) != len(si.on_wait):
                si.on_wait = keep
            if len(keep) > 1 and tname != "InstDrain":
                bad.append((inst.name, tname,
                            [w.ant_name for w in keep]))
    assert not bad, f"multi-wait compute instructions remain: {bad}"
    return nc


def _prep_weights(spline_coeff, spline_scaling):
    # C'[m,i,j] = (1/(6h^3)) * sum_g w[m-g] * coeff[i,j,g] * scaling[i,j]
    h = 2.0 / GRID_SIZE
    c = (spline_coeff.astype(np.float64)
         * spline_scaling.astype(np.float64)[:, :, None])  # [i, j, g]
    cp = np.zeros((NM, IN_DIM, OUT_DIM), np.float64)
    for m in range(NM):
        for g in range(max(0, m - 4), m + 1):
            cp[m] += _W_BINOM[m - g] * c[:, :, g]
    cp *= 1.0 / (6.0 * h ** 3)
    # partition-major swizzle: CWH[p, m, ic, j] = cp[m, ic*128 + p, j]
    cwh = (cp.reshape(NM, NIC, P, OUT_DIM)
           .transpose(2, 0, 1, 3)
           .reshape(P, NM * NCOL)
           .astype(np.float32))
    return np.ascontiguousarray(cwh)


def _swizzle_x(xc):
    # X[p, ic*BC + b] = xc[b, ic*128 + p]
    return np.ascontiguousarray(
        xc.T.reshape(NIC, P, BC).transpose(1, 0, 2).reshape(P, NCOL))


def _run(inputs, trace=False, mm_dtype_name="float32r"):
    from concourse.bass_utils import run_bass_kernel_spmd

    key = "v2"
    if key not in _cached:
        _cached[key] = _build_nc()
    nc = _cached[key]

    x = np.asarray(inputs["x"], np.float32)
    cw = _prep_weights(np.asarray(inputs["spline_coeff"]),
                       np.asarray(inputs["spline_scaling"]))
    in_maps = []
    for c in range(N_CORES):
        xc = _swizzle_x(x[c * BC:(c + 1) * BC, :])
        in_maps.append({"xt": xc, "cw": cw})
    res = run_bass_kernel_spmd(nc, in_maps, list(range(N_CORES)),
                               trace=trace)
    outp = np.concatenate(
        [np.asarray(res.results[c]["out"]).astype(np.float32)
         for c in range(N_CORES)], axis=0)
    return outp, res


def kernel(**inputs):
    outp, _ = _run(inputs, trace=False)
    return outp
